# revision 19
# baseline (speedup 1.0000x reference)
"""AvgNeighborSimEncoder kernel for 8 Trainium2 NeuronCores.

Math: for each node, avg over unordered neighbor pairs (m<n) of sim[i_m, i_n]
  = (v^T S v - sum_m S[i_m,i_m]) / 2 / (deg*(deg-1)/2),  v = neighbor count vec.
Then idx = int(avg*1000) (round-to-nearest on this backend); out = emb[idx].

Implementation (node-partitioned across 8 cores, no collectives):
  - Core k owns rna nodes [375k, 375(k+1)) and a ~188-node dis shard.
  - Host buckets the core's edges by (slot-tile, 128-col-tile); the device
    densifies the count matrix A via one-hot matmuls (128 edges/chunk), then
    PE-transposes it to get the matmul operand layout AT.
  - T = A @ S computed as an fp16 hi/lo split of S*2^12 (integer counts are
    exact in fp16; |S*2^12 - hi - lo| <= ~2^-10 => fp32-grade after unscale).
  - quad = rowsum(A * T); diag/deg via a narrow matmul against [diag(S)|1].
  - avg = pair/npair via reciprocal-table gather + 2 exact-residual
    refinement steps (npair is a small integer).
  - emb rows gathered on-device by indirect DMA; host only reassembles shards.
  - All DRAM inputs are pre-tiled on the host into SBUF-image layouts so every
    DMA is a fat contiguous-per-partition transfer.
"""
import sys
sys.path.insert(0, "/opt/trn_rl_repo")

import numpy as np

NUM_RNA = 3000
NUM_DIS = 1500
EMB_ROWS = 4500
EMB_DIM = 128
N_CORES = 8

SC = 2.0 ** 12             # plane pre-scale (power of 2 commutes with rounding)
UNSCALE_PAIR = 2.0 ** -13  # 0.5 * 2^-12

RNA_PER_CORE = NUM_RNA // N_CORES      # 375
RNA_SLOT_TILES = 3                     # 375 -> 384 slots
DIS_SLOT_TILES = 2                     # <=188 -> 256 slots
KP_DS = 1536                           # padded ds dim (12 K-tiles)
KP_MS = 3072                           # padded ms dim (24 K-tiles)
KT_DS = KP_DS // 128
KT_MS = KP_MS // 128
NB_DS = KP_DS // 512                   # 512-wide col blocks (T matmul tiling)
NB_MS = KP_MS // 512
HALF_MS = KP_MS // 2
N_ST = RNA_SLOT_TILES + DIS_SLOT_TILES  # 5 slot tiles total
RECIP_N = 16384

_DIS_SIZES = [188, 188, 188, 188, 187, 187, 187, 187]
_DIS_STARTS = np.concatenate([[0], np.cumsum(_DIS_SIZES)])[:-1]


def _split_planes(S, kp):
    """Pad sim matrix to [kp,kp]; split S*2^12 into fp16 hi+lo planes."""
    n = S.shape[0]
    Sp = np.zeros((kp, kp), dtype=np.float32)
    Sp[:n, :n] = S
    hi = (Sp * SC).astype(np.float16)
    lo = (Sp * SC - hi.astype(np.float32)).astype(np.float16)
    return hi, lo


def _ds_image(hi, lo):
    """SBUF image [128, 2, KT_DS, KP_DS]: [p, plane, kt, m] = plane[kt*128+p, m]."""
    img = np.empty((128, 2, KT_DS, KP_DS), dtype=np.float16)
    for pl, M in ((0, hi), (1, lo)):
        img[:, pl] = M.reshape(KT_DS, 128, KP_DS).transpose(1, 0, 2)
    return img.reshape(128, 2 * KT_DS * KP_DS)


def _ms_image(hi, lo):
    """[KT_MS*2*128, 2*HALF_MS]: row (kt*2+half)*128+p = hi|lo halves packed."""
    img = np.empty((KT_MS, 2, 128, 2 * HALF_MS), dtype=np.float16)
    h4 = hi.reshape(KT_MS, 128, 2, HALF_MS)
    l4 = lo.reshape(KT_MS, 128, 2, HALF_MS)
    for half in range(2):
        img[:, half, :, :HALF_MS] = h4[:, :, half]
        img[:, half, :, HALF_MS:] = l4[:, :, half]
    return img.reshape(KT_MS * 2 * 128, 2 * HALF_MS)


def _dd_image(hi, lo, kt):
    """[128, 2, kt, 2]: [p, plane, k, 0] = diag(plane)[k*128+p], [...,1] = 1 (hi)."""
    img = np.zeros((128, 2, kt, 2), dtype=np.float16)
    img[:, 0, :, 0] = np.diagonal(hi).reshape(kt, 128).T
    img[:, 1, :, 0] = np.diagonal(lo).reshape(kt, 128).T
    img[:, 0, :, 1] = 1.0
    return img.reshape(128, 2 * kt * 2)


def _bucketize(slots, cols, n_st, n_ct):
    st = slots // 128
    ct = cols // 128
    out = {}
    for s in range(n_st):
        for c in range(n_ct):
            m = (st == s) & (ct == c)
            out[(s, c)] = (slots[m] - 128 * s, cols[m] - 128 * c)
    return out


def _pack_edges(per_core_buckets, n_st, n_ct):
    """Unified (max-over-cores) chunk counts; pack ids into [128, C] (pad -1)."""
    schedule = []
    for s in range(n_st):
        for c in range(n_ct):
            mx = max(len(b[(s, c)][0]) for b in per_core_buckets)
            if mx == 0:
                continue
            schedule.append((s, c, -(-mx // 128)))
    c_total = sum(n for _, _, n in schedule)
    slot_arrs, col_arrs = [], []
    for b in per_core_buckets:
        sa = np.full((128, c_total), -1, dtype=np.int32)
        ca = np.full((128, c_total), -1, dtype=np.int32)
        off = 0
        for s, c, n_chunks in schedule:
            sl, co = b[(s, c)]
            k = len(sl)
            fs = np.full(n_chunks * 128, -1, dtype=np.int32)
            fc = np.full(n_chunks * 128, -1, dtype=np.int32)
            fs[:k] = sl
            fc[:k] = co
            sa[:, off:off + n_chunks] = fs.reshape(n_chunks, 128).T
            ca[:, off:off + n_chunks] = fc.reshape(n_chunks, 128).T
            off += n_chunks
        slot_arrs.append(sa)
        col_arrs.append(ca)
    return schedule, slot_arrs, col_arrs


def _build_program(sched_rna, c1, sched_dis, c2, n_reps=1, skip=()):
    import concourse.bass as bass
    import concourse.tile as tile
    from concourse import bacc, mybir
    from concourse.masks import make_identity

    f32, f16, i32 = mybir.dt.float32, mybir.dt.float16, mybir.dt.int32
    AOp = mybir.AluOpType

    nc = bacc.Bacc("TRN2", target_bir_lowering=False)

    CE = 2 * (c1 + c2)
    ds_img_p = nc.declare_dram_parameter("ds_img", [128, 2 * KT_DS * KP_DS], f16, isOutput=False)
    ms_img_p = nc.declare_dram_parameter("ms_img", [KT_MS * 2 * 128, 2 * HALF_MS], f16, isOutput=False)
    ddd_p = nc.declare_dram_parameter("dd_ds", [128, 2 * KT_DS * 2], f16, isOutput=False)
    ddm_p = nc.declare_dram_parameter("dd_ms", [128, 2 * KT_MS * 2], f16, isOutput=False)
    edges_p = nc.declare_dram_parameter("edges", [128, CE], i32, isOutput=False)
    recip_p = nc.declare_dram_parameter("recip", [RECIP_N, 1], f32, isOutput=False)
    emb_p = nc.declare_dram_parameter("emb", [EMB_ROWS, EMB_DIM], f32, isOutput=False)
    out_p = nc.declare_dram_parameter("out_emb", [N_ST * 128, EMB_DIM], f32, isOutput=True)

    for _rep in range(n_reps):
      with tile.TileContext(nc) as tc:
        with (
            tc.tile_pool(name="const", bufs=1) as cp,
            tc.tile_pool(name="mats", bufs=1) as mp,
            tc.tile_pool(name="work", bufs=4) as wp,
            tc.tile_pool(name="stream", bufs=4) as sp,
        ):
            # ---------- constants ----------
            ident = cp.tile([128, 128], f16)
            make_identity(nc, ident[:])
            iota = cp.tile([128, 128], i32)
            nc.gpsimd.iota(iota[:], pattern=[[1, 128]], base=0, channel_multiplier=0)

            t_edges = cp.tile([128, CE], i32)
            nc.sync.dma_start(out=t_edges[:], in_=edges_p[:])
            t_rslot = t_edges[:, 0:c1]
            t_rcol = t_edges[:, c1:2 * c1]
            t_dslot = t_edges[:, 2 * c1:2 * c1 + c2]
            t_dcol = t_edges[:, 2 * c1 + c2:2 * c1 + 2 * c2]

            # ds planes resident: [128, 2(plane), KT_DS, KP_DS]
            t_ds = mp.tile([128, 2, KT_DS, KP_DS], f16)
            nc.sync.dma_start(
                out=t_ds[:],
                in_=ds_img_p[:].rearrange("p (a b c) -> p a b c", a=2, b=KT_DS))
            t_ddd = cp.tile([128, 2, KT_DS, 2], f16)
            nc.sync.dma_start(
                out=t_ddd[:],
                in_=ddd_p[:].rearrange("p (a b c) -> p a b c", a=2, b=KT_DS))
            t_ddm = cp.tile([128, 2, KT_MS, 2], f16)
            nc.sync.dma_start(
                out=t_ddm[:],
                in_=ddm_p[:].rearrange("p (a b c) -> p a b c", a=2, b=KT_MS))

            # count matrices: elementwise layout + transposed (matmul lhsT) layout
            t_A = mp.tile([128, RNA_SLOT_TILES, KP_DS], f16)
            t_AT = mp.tile([128, KT_DS, RNA_SLOT_TILES * 128], f16)
            t_B = mp.tile([128, DIS_SLOT_TILES, KP_MS], f16)
            t_BT = mp.tile([128, KT_MS, DIS_SLOT_TILES * 128], f16)

            # ---------- densify via one-hot matmuls (128-edge chunks) ----------
            def densify(pdens, schedule, t_slot_ids, t_col_ids, t_dst, t_dstT,
                        n_slot_tiles, n_ktiles, off0):
                off = off0
                for (s, c, n_chunks) in schedule:
                    ps_cnt = pdens.tile([128, 128], f32, space="PSUM", tag="ps_cnt")
                    for j in range(n_chunks):
                        ci = off + j
                        oh_r = wp.tile([128, 128], f16, tag="oh_r")
                        oh_d = wp.tile([128, 128], f16, tag="oh_d")
                        nc.vector.tensor_tensor(
                            out=oh_r[:],
                            in0=t_slot_ids[:, ci:ci + 1].to_broadcast([128, 128]),
                            in1=iota[:], op=AOp.is_equal)
                        nc.vector.tensor_tensor(
                            out=oh_d[:],
                            in0=t_col_ids[:, ci:ci + 1].to_broadcast([128, 128]),
                            in1=iota[:], op=AOp.is_equal)
                        nc.tensor.matmul(out=ps_cnt[:], lhsT=oh_r[:], rhs=oh_d[:],
                                         start=(j == 0), stop=(j == n_chunks - 1))
                    nc.vector.tensor_copy(out=t_dst[:, s, 128 * c:128 * (c + 1)], in_=ps_cnt[:])
                    off += n_chunks
                # fill untouched (empty-bucket) regions with zeros
                covered = {(s, c) for s, c, _ in schedule}
                for s in range(n_slot_tiles):
                    for c in range(n_ktiles):
                        if (s, c) not in covered:
                            nc.vector.memset(t_dst[:, s, 128 * c:128 * (c + 1)], 0.0)
                # transpose to lhsT layout
                for s in range(n_slot_tiles):
                    for kt in range(n_ktiles):
                        ps_tr = pdens.tile([128, 128], f16, space="PSUM", tag="ps_tr")
                        nc.tensor.transpose(out=ps_tr[:], in_=t_dst[:, s, 128 * kt:128 * (kt + 1)],
                                            identity=ident[:])
                        nc.vector.tensor_copy(out=t_dstT[:, kt, 128 * s:128 * (s + 1)], in_=ps_tr[:])

            if "densify" not in skip:
                with tc.tile_pool(name="psum_dens", bufs=2, space="PSUM") as pdens:
                    densify(pdens, sched_rna, t_rslot, t_rcol, t_A, t_AT,
                            RNA_SLOT_TILES, KT_DS, 0)
                    densify(pdens, sched_dis, t_dslot, t_dcol, t_B, t_BT,
                            DIS_SLOT_TILES, KT_MS, 0)
            else:
                nc.vector.memset(t_A[:], 0.0)
                nc.vector.memset(t_AT[:], 0.0)
                nc.vector.memset(t_B[:], 0.0)
                nc.vector.memset(t_BT[:], 0.0)

            # ---------- per-node accumulators [128, N_ST] ----------
            t_quad = mp.tile([128, N_ST], f32)
            t_diag = mp.tile([128, N_ST], f32)
            t_deg = mp.tile([128, N_ST], f32)
            nc.vector.memset(t_quad[:], 0.0)

            # ---------- rna side: T1 = A @ ds (scaled), quad/diag/deg ----------
            prna_cm = tc.tile_pool(name="psum_rna", bufs=2, space="PSUM")
            prna = prna_cm.__enter__()
            for s in range(RNA_SLOT_TILES if "t1" not in skip else 0):
                for nb in range(NB_DS):
                    ps_t = prna.tile([128, 512], f32, space="PSUM", tag="ps_t")
                    n_mm = 0
                    for kt in range(KT_DS):
                        for pl in range(2):
                            nc.tensor.matmul(
                                out=ps_t[:],
                                lhsT=t_AT[:, kt, 128 * s:128 * (s + 1)],
                                rhs=t_ds[:, pl, kt, 512 * nb:512 * (nb + 1)],
                                start=(n_mm == 0), stop=(n_mm == 2 * KT_DS - 1))
                            n_mm += 1
                    prod = wp.tile([128, 512], f32, tag="prod")
                    nc.vector.tensor_tensor(out=prod[:], in0=ps_t[:],
                                            in1=t_A[:, s, 512 * nb:512 * (nb + 1)], op=AOp.mult)
                    part = wp.tile([128, 1], f32, tag="part")
                    nc.vector.tensor_reduce(out=part[:], in_=prod[:],
                                            axis=mybir.AxisListType.X, op=AOp.add)
                    nc.vector.tensor_tensor(out=t_quad[:, s:s + 1], in0=t_quad[:, s:s + 1],
                                            in1=part[:], op=AOp.add)
                ps_dd = prna.tile([128, 2], f32, space="PSUM", tag="ps_dd")
                n_mm = 0
                for kt in range(KT_DS):
                    for pl in range(2):
                        nc.tensor.matmul(
                            out=ps_dd[:],
                            lhsT=t_AT[:, kt, 128 * s:128 * (s + 1)],
                            rhs=t_ddd[:, pl, kt, :],
                            start=(n_mm == 0), stop=(n_mm == 2 * KT_DS - 1))
                        n_mm += 1
                nc.vector.tensor_copy(out=t_diag[:, s:s + 1], in_=ps_dd[:, 0:1])
                nc.vector.tensor_copy(out=t_deg[:, s:s + 1], in_=ps_dd[:, 1:2])
            if "t1" in skip:
                nc.vector.memset(t_diag[:, :RNA_SLOT_TILES], 0.0)
                nc.vector.memset(t_deg[:, :RNA_SLOT_TILES], 0.0)
            prna_cm.__exit__(None, None, None)

            # ---------- dis side: T2 = B @ ms (ms streamed), quad/diag/deg ----------
            pdis_cm = tc.tile_pool(name="psum_dis", bufs=1, space="PSUM")
            pdis = pdis_cm.__enter__()
            NBH = NB_MS // 2
            for half in range(2 if "t2" not in skip else 0):
                ps_t2 = [pdis.tile([128, 512], f32, space="PSUM",
                                   tag=f"ps_t2_{i}", name=f"ps_t2_{i}", bufs=1)
                         for i in range(DIS_SLOT_TILES * NBH)]
                n_mm = [0] * len(ps_t2)
                for kt in range(KT_MS):
                    m_t = sp.tile([128, 2 * HALF_MS], f16, tag="m_t")
                    nc.sync.dma_start(
                        out=m_t[:],
                        in_=ms_img_p[(kt * 2 + half) * 128:(kt * 2 + half + 1) * 128, :])
                    for s in range(DIS_SLOT_TILES):
                        for nb in range(NBH):
                            i = s * NBH + nb
                            for pl in range(2):
                                nc.tensor.matmul(
                                    out=ps_t2[i][:],
                                    lhsT=t_BT[:, kt, 128 * s:128 * (s + 1)],
                                    rhs=m_t[:, pl * HALF_MS + 512 * nb: pl * HALF_MS + 512 * (nb + 1)],
                                    start=(n_mm[i] == 0), stop=(n_mm[i] == 2 * KT_MS - 1))
                                n_mm[i] += 1
                for s in range(DIS_SLOT_TILES):
                    for nb in range(NBH):
                        i = s * NBH + nb
                        g_nb = half * NBH + nb
                        sq = RNA_SLOT_TILES + s
                        prod = wp.tile([128, 512], f32, tag="prod")
                        nc.vector.tensor_tensor(out=prod[:], in0=ps_t2[i][:],
                                                in1=t_B[:, s, 512 * g_nb:512 * (g_nb + 1)],
                                                op=AOp.mult)
                        part = wp.tile([128, 1], f32, tag="part")
                        nc.vector.tensor_reduce(out=part[:], in_=prod[:],
                                                axis=mybir.AxisListType.X, op=AOp.add)
                        nc.vector.tensor_tensor(out=t_quad[:, sq:sq + 1],
                                                in0=t_quad[:, sq:sq + 1],
                                                in1=part[:], op=AOp.add)
            for s in range(DIS_SLOT_TILES if "t2" not in skip else 0):
                sq = RNA_SLOT_TILES + s
                ps_dd = pdis.tile([128, 2], f32, space="PSUM", tag="ps_dd")
                n_mm = 0
                for kt in range(KT_MS):
                    for pl in range(2):
                        nc.tensor.matmul(
                            out=ps_dd[:],
                            lhsT=t_BT[:, kt, 128 * s:128 * (s + 1)],
                            rhs=t_ddm[:, pl, kt, :],
                            start=(n_mm == 0), stop=(n_mm == 2 * KT_MS - 1))
                        n_mm += 1
                nc.vector.tensor_copy(out=t_diag[:, sq:sq + 1], in_=ps_dd[:, 0:1])
                nc.vector.tensor_copy(out=t_deg[:, sq:sq + 1], in_=ps_dd[:, 1:2])
            if "t2" in skip:
                nc.vector.memset(t_diag[:, RNA_SLOT_TILES:], 0.0)
                nc.vector.memset(t_deg[:, RNA_SLOT_TILES:], 0.0)
            pdis_cm.__exit__(None, None, None)

            # ---------- tail ----------
            t_pair = mp.tile([128, N_ST], f32)
            nc.vector.tensor_tensor(out=t_pair[:], in0=t_quad[:], in1=t_diag[:], op=AOp.subtract)
            nc.vector.tensor_scalar_mul(t_pair[:], t_pair[:], UNSCALE_PAIR)

            t_np = mp.tile([128, N_ST], f32)
            nc.vector.tensor_tensor(out=t_np[:], in0=t_deg[:], in1=t_deg[:], op=AOp.mult)
            nc.vector.tensor_tensor(out=t_np[:], in0=t_np[:], in1=t_deg[:], op=AOp.subtract)
            nc.vector.tensor_scalar_mul(t_np[:], t_np[:], 0.5)
            nc.vector.tensor_scalar_max(t_np[:], t_np[:], 1.0)

            t_npi = mp.tile([128, N_ST], i32)
            t_npc = mp.tile([128, N_ST], f32)
            nc.vector.tensor_scalar_min(t_npc[:], t_np[:], float(RECIP_N - 1))
            nc.vector.tensor_copy(out=t_npi[:], in_=t_npc[:])

            t_r = mp.tile([128, N_ST], f32)
            for s in range(N_ST):
                nc.gpsimd.indirect_dma_start(
                    out=t_r[:, s:s + 1], out_offset=None, in_=recip_p[:],
                    in_offset=bass.IndirectOffsetOnAxis(ap=t_npi[:, s:s + 1], axis=0))

            t_q = mp.tile([128, N_ST], f32)
            t_t = mp.tile([128, N_ST], f32)
            t_e = mp.tile([128, N_ST], f32)
            nc.vector.tensor_tensor(out=t_q[:], in0=t_pair[:], in1=t_r[:], op=AOp.mult)
            for _ in range(2):
                nc.vector.tensor_tensor(out=t_t[:], in0=t_q[:], in1=t_np[:], op=AOp.mult)
                nc.vector.tensor_tensor(out=t_e[:], in0=t_pair[:], in1=t_t[:], op=AOp.subtract)
                nc.vector.tensor_tensor(out=t_e[:], in0=t_e[:], in1=t_r[:], op=AOp.mult)
                nc.vector.tensor_tensor(out=t_q[:], in0=t_q[:], in1=t_e[:], op=AOp.add)

            t_y = mp.tile([128, N_ST], f32)
            nc.vector.tensor_scalar_mul(t_y[:], t_q[:], 1000.0)
            nc.vector.tensor_scalar_max(t_y[:], t_y[:], 0.0)
            # round-to-nearest cast, matching the reference's on-device cast
            t_idx = mp.tile([128, N_ST], i32)
            nc.vector.tensor_copy(out=t_idx[:], in_=t_y[:])

            # ---------- gather emb rows, write out ----------
            t_out = mp.tile([128, N_ST, EMB_DIM], f32)
            for s in range(N_ST):
                nc.gpsimd.indirect_dma_start(
                    out=t_out[:, s, :], out_offset=None, in_=emb_p[:],
                    in_offset=bass.IndirectOffsetOnAxis(ap=t_idx[:, s:s + 1], axis=0))
            nc.sync.dma_start(out=out_p[:].rearrange("(n p) m -> p n m", p=128), in_=t_out[:])

    nc.compile()
    return nc


_PROGRAM_CACHE = {}


def _prepare(inputs, n_reps=1, skip=()):
    assoc = np.asarray(inputs["associations"], dtype=np.int32)
    ms = np.asarray(inputs["ms"], dtype=np.float32)
    ds = np.asarray(inputs["ds"], dtype=np.float32)
    emb = np.asarray(inputs["emb"], dtype=np.float32)

    rna = assoc[0]
    dis = assoc[1] - NUM_RNA

    ds_hi, ds_lo = _split_planes(ds, KP_DS)
    ms_hi, ms_lo = _split_planes(ms, KP_MS)
    ds_img = _ds_image(ds_hi, ds_lo)
    ms_img = _ms_image(ms_hi, ms_lo)
    dd_ds = _dd_image(ds_hi, ds_lo, KT_DS)
    dd_ms = _dd_image(ms_hi, ms_lo, KT_MS)

    recip = np.ones((RECIP_N, 1), dtype=np.float32)
    recip[1:, 0] = (1.0 / np.arange(1, RECIP_N, dtype=np.float64)).astype(np.float32)

    rna_buckets, dis_buckets = [], []
    for k in range(N_CORES):
        m1 = (rna // RNA_PER_CORE) == k
        rna_buckets.append(_bucketize(rna[m1] - RNA_PER_CORE * k, dis[m1],
                                      RNA_SLOT_TILES, KT_DS))
        d0 = _DIS_STARTS[k]
        m2 = (dis >= d0) & (dis < d0 + _DIS_SIZES[k])
        dis_buckets.append(_bucketize(dis[m2] - d0, rna[m2], DIS_SLOT_TILES, KT_MS))
    sched_rna, r_slots, r_cols = _pack_edges(rna_buckets, RNA_SLOT_TILES, KT_DS)
    sched_dis, d_slots, d_cols = _pack_edges(dis_buckets, DIS_SLOT_TILES, KT_MS)
    c1 = r_slots[0].shape[1]
    c2 = d_slots[0].shape[1]

    key = (tuple(sched_rna), c1, tuple(sched_dis), c2, n_reps, tuple(skip))
    nc = _PROGRAM_CACHE.get(key)
    if nc is None:
        nc = _build_program(sched_rna, c1, sched_dis, c2, n_reps=n_reps, skip=skip)
        _PROGRAM_CACHE[key] = nc

    in_maps = []
    for k in range(N_CORES):
        edges = np.concatenate([r_slots[k], r_cols[k], d_slots[k], d_cols[k]], axis=1)
        in_maps.append({
            "ds_img": ds_img.view(np.uint16), "ms_img": ms_img.view(np.uint16),
            "dd_ds": dd_ds.view(np.uint16), "dd_ms": dd_ms.view(np.uint16),
            "edges": np.ascontiguousarray(edges),
            "recip": recip, "emb": emb,
        })
    return nc, in_maps


def _unshard(results):
    out = np.empty((EMB_ROWS, EMB_DIM), dtype=np.float32)
    for k in range(N_CORES):
        o = results[k]["out_emb"]
        out[RNA_PER_CORE * k: RNA_PER_CORE * (k + 1)] = o[:RNA_PER_CORE]
        d0 = _DIS_STARTS[k]
        nd = _DIS_SIZES[k]
        out[NUM_RNA + d0: NUM_RNA + d0 + nd] = \
            o[RNA_SLOT_TILES * 128: RNA_SLOT_TILES * 128 + nd]
    return out


def kernel(**inputs):
    from concourse.bass_utils import run_bass_kernel_spmd
    nc, in_maps = _prepare(inputs)
    res = run_bass_kernel_spmd(nc, in_maps, list(range(N_CORES)))
    return _unshard(res.results)
